# revision 37
# baseline (speedup 1.0000x reference)
"""Additive attention (Bahdanau-style) on 8 TRN2 NeuronCores.

Data-parallel over batch: each core handles 32 of the 256 batch items.
Per core (rows = 32*196 = 6272, Dv = 2048, A = 512):

  vaT   = W1^T @ values^T                                [A, rows]  (PE)
  tanhT = tanh(vaT + qaT[:, b(r)])     (qaT bias folded into ScalarE)
  s     = v^T @ tanhT                                    [1, rows]  (PE)
  e     = exp(s + bv)     (no max subtraction: |s| <= ||v||_1 ~ 11) (ScalarE)
  Z_b   = sum_n e, attn = e/Z        (incremental, per batch item)  (VectorE)
  ctx   = blockdiag(attn)^T @ values  (4x PE column-group packing)

Compute dtype is bf16 (host-cast inputs); accumulation is fp32 in PSUM.
values is supplied in both layouts ([rows, Dv] for the context matmul and
[Dv, rows] for the W1 matmul) as bf16 - same total HBM bytes as one fp32
copy. A 3-stage software pipeline (scores trail the W1 matmul by one chunk,
context by two) keeps the TensorEngine stream dense.
"""

import numpy as np
import ml_dtypes

from contextlib import ExitStack

from concourse import bacc, tile, mybir
from concourse.bass_utils import run_bass_kernel_spmd

F32 = mybir.dt.float32
BF16 = mybir.dt.bfloat16
FP8 = mybir.dt.float8e4
BF16_NP = ml_dtypes.bfloat16
FP8_NP = ml_dtypes.float8_e4m3fn
W1_SCALE = 256.0

NCORES = 8
B = 256
BSH = B // NCORES          # 32 batch items per core
N = 196                    # sequence length
ROWS = BSH * N             # 6272
DV = 2048
A = 512
DQ = 512

CHUNK = 512                # rows per pipeline chunk (4 partition tiles)
NKT = ROWS // 128          # 49
CHUNKS = [(i * CHUNK, min(CHUNK, ROWS - i * CHUNK))
          for i in range((ROWS + CHUNK - 1) // CHUNK)]

_CACHE = {}


def _bsegs(rc, nr):
    """Split chunk rows [rc, rc+nr) at batch-item boundaries.
    Yields (local_start, local_end, b)."""
    out = []
    r = rc
    while r < rc + nr:
        b = r // N
        e = min((b + 1) * N, rc + nr)
        out.append((r - rc, e - rc, b))
        r = e
    return out


def _build_nc():
    nc = bacc.Bacc("TRN2", target_bir_lowering=False, debug=False,
                   num_devices=NCORES)

    vals = nc.dram_tensor("vals", [ROWS, DV], BF16, kind="ExternalInput").ap()
    valsT = nc.dram_tensor("valsT", [DV, ROWS], FP8, kind="ExternalInput").ap()
    qT = nc.dram_tensor("qT", [DQ, BSH], BF16, kind="ExternalInput").ap()
    w1 = nc.dram_tensor("w1", [DV, A], FP8, kind="ExternalInput").ap()
    w2 = nc.dram_tensor("w2", [DQ, A], BF16, kind="ExternalInput").ap()
    b12T = nc.dram_tensor("b12T", [128, A // 128], F32, kind="ExternalInput").ap()
    vv = nc.dram_tensor("vv", [A, 1], BF16, kind="ExternalInput").ap()
    bvv = nc.dram_tensor("bvv", [1, 1], F32, kind="ExternalInput").ap()
    maskw = nc.dram_tensor("maskw", [128, NKT, BSH], BF16,
                           kind="ExternalInput").ap()
    out_ctx = nc.dram_tensor("out_ctx", [BSH, DV], F32, kind="ExternalOutput").ap()
    out_attn = nc.dram_tensor("out_attn", [BSH, N], F32, kind="ExternalOutput").ap()

    TANH = mybir.ActivationFunctionType.Tanh
    EXP = mybir.ActivationFunctionType.Exp
    NM = A // 128   # 4 m-tiles of the attention dim
    NKC = DV // 128  # 16 contraction tiles

    with tile.TileContext(nc) as tc, ExitStack() as ctx:
        consts = ctx.enter_context(tc.tile_pool(name="consts", bufs=1))
        ppersist = ctx.enter_context(tc.tile_pool(name="ppersist", bufs=1, space="PSUM"))
        pva = ctx.enter_context(tc.tile_pool(name="pva", bufs=3, space="PSUM"))
        psmall = ctx.enter_context(tc.tile_pool(name="psmall", bufs=2, space="PSUM"))
        vnat_pool = ctx.enter_context(tc.tile_pool(name="vnat", bufs=3))
        vt_pool = ctx.enter_context(tc.tile_pool(name="vt", bufs=3))
        tanh_pool = ctx.enter_context(tc.tile_pool(name="tanh", bufs=2))
        acol_pool = ctx.enter_context(tc.tile_pool(name="acol", bufs=4))
        ecol_pool = ctx.enter_context(tc.tile_pool(name="ecol", bufs=3))
        pvb_pool = ctx.enter_context(tc.tile_pool(name="pvb", bufs=3))
        dram_pool = ctx.enter_context(tc.tile_pool(name="dram", bufs=1, space="DRAM"))

        # ---- constants / weights (ACT HWDGE ring; small ones first so
        # the query projection and first W1 k-tiles are ready ASAP) ------
        qT_sb = consts.tile([128, DQ // 128, BSH], BF16)
        nc.scalar.dma_start(out=qT_sb[:], in_=qT.rearrange("(k p) b -> p k b", p=128))
        w2_sb = consts.tile([128, DQ // 128, A], BF16)
        nc.scalar.dma_start(out=w2_sb[:], in_=w2.rearrange("(k p) a -> p k a", p=128))
        b12T_sb = consts.tile([128, NM], F32)
        nc.scalar.dma_start(out=b12T_sb[:], in_=b12T)
        w1_sb = consts.tile([128, NKC, A], FP8)
        w1_v = w1.rearrange("(k p) a -> p k a", p=128)
        for q in range(4):
            kq = NKC // 4
            nc.scalar.dma_start(out=w1_sb[:, q * kq:(q + 1) * kq, :],
                                in_=w1_v[:, q * kq:(q + 1) * kq, :])
        v_sb = consts.tile([128, NM, 1], BF16)
        nc.scalar.dma_start(out=v_sb[:], in_=vv.rearrange("(k p) o -> p k o", p=128))
        bv_sb = consts.tile([1, 1], F32)
        nc.scalar.dma_start(out=bv_sb[:], in_=bvv)
        mask_sb = consts.tile([128, NKT, BSH], BF16)
        nc.scalar.dma_start(out=mask_sb[:], in_=maskw)
        id1 = consts.tile([1, 1], F32)
        nc.vector.memset(id1[:], 1.0)
        # warmup fodder: keeps the PE busy (and the HAM un-throttled) while
        # the first weight/value DMAs are still in flight
        wsrc = consts.tile([128, 512], BF16)
        nc.vector.memset(wsrc[:], 0.0)

        e_row = consts.tile([1, ROWS], F32)
        escratch = dram_pool.tile([ROWS], F32)
        ctx_sb = consts.tile([128, DV // NM], F32)
        zrow = consts.tile([1, BSH], F32)
        rzrow = consts.tile([1, BSH], F32)
        rz_col4 = consts.tile([128, 1], F32)
        qaT_sb = consts.tile([128, NM, BSH], F32)

        # context accumulator: column group j holds dv quarter j on
        # partitions [32j, 32j+32)
        psum_ctx = ppersist.tile([128, DV // NM], F32)

        # ---- PE warmup while startup DMAs land ------------------------
        for w in range(10):
            pwarm = psmall.tile([128, 512], F32, tag="sm", name="pwarm")
            nc.tensor.matmul(pwarm[:, 0:384], wsrc[:, 0:128], wsrc[:, 0:384],
                             start=True, stop=True)

        # ---- query projection qaT = W2^T @ q^T + (b1+b2) --------------
        pqa = psmall.tile([128, NM, BSH], F32, tag="sm")
        for m in range(NM):
            for j in range(DQ // 128):
                nc.tensor.matmul(pqa[:, m, :], w2_sb[:, j, m * 128:(m + 1) * 128],
                                 qT_sb[:, j, :], start=(j == 0),
                                 stop=(j == DQ // 128 - 1))
        for m in range(NM):
            nc.vector.tensor_scalar_add(qaT_sb[:, m, :], pqa[:, m, :],
                                        b12T_sb[:, m:m + 1])

        # ---- 3-stage pipelined main loop ------------------------------
        state = {}

        def stage_load(c):
            rc, nr = CHUNKS[c]
            vt = vt_pool.tile([128, NKC, CHUNK], FP8, name="vt")
            vT_v = valsT.rearrange("(k p) r -> p k r", p=128)
            # split so each quarter's matmuls can start as soon as it lands
            nsplit = 4 if c == 0 else 2
            kq = NKC // nsplit
            for q in range(nsplit):
                nc.sync.dma_start(out=vt[:, q * kq:(q + 1) * kq, :nr],
                                  in_=vT_v[:, q * kq:(q + 1) * kq, rc:rc + nr])
            vnat = vnat_pool.tile([128, CHUNK // 128, DV], BF16, name="vnat")
            nc.scalar.dma_start(
                out=vnat[:, :nr // 128, :],
                in_=vals[rc:rc + nr, :].rearrange("(t p) d -> p t d", p=128))
            state[c] = {"vt": vt, "vnat": vnat}

        def stage_va(c):
            rc, nr = CHUNKS[c]
            st = state[c]
            vt = st["vt"]
            tanhT = [tanh_pool.tile([128, CHUNK], BF16, tag=f"tanh{m}",
                                    name=f"tanhT{m}") for m in range(NM)]
            for m in range(NM):
                pv = pva.tile([128, CHUNK], F32, name="pv")
                for kp in range(NKC // 2):
                    nc.tensor.matmul(pv[:, :nr],
                                     w1_sb[:, 2 * kp:2 * kp + 2,
                                           m * 128:(m + 1) * 128],
                                     vt[:, 2 * kp:2 * kp + 2, :nr],
                                     start=(kp == 0), stop=(kp == NKC // 2 - 1),
                                     perf_mode=mybir.MatmulPerfMode.DoubleRow)
                for (s0, s1, b) in _bsegs(rc, nr):
                    nc.scalar.activation(tanhT[m][:, s0:s1], pv[:, s0:s1], TANH,
                                         bias=qaT_sb[:, m, b:b + 1],
                                         scale=1.0 / W1_SCALE)
            st["tanhT"] = tanhT

        def stage_scores(c):
            rc, nr = CHUNKS[c]
            st = state[c]
            tanhT = st["tanhT"]
            gs = slice(rc, rc + nr)
            psc = psmall.tile([1, CHUNK], F32, tag="sm", name="psc")
            for j in range(NM):
                nc.tensor.matmul(psc[:, :nr], v_sb[:, j, :], tanhT[j][:, :nr],
                                 start=(j == 0), stop=(j == NM - 1))
            nc.scalar.activation(e_row[0:1, gs], psc[:, :nr], EXP,
                                 bias=bv_sb[0:1, 0:1])
            if c < len(CHUNKS) - 2:
                # park e in DRAM so it can come back partition-major
                nc.sync.dma_start(out=escratch[gs], in_=e_row[0:1, gs])
            nt = nr // 128
            ecol = ecol_pool.tile([128, CHUNK // 128], F32, name="ecol")
            if c >= len(CHUNKS) - 2:
                # tail chunks: PE-transpose e into columns to skip the DRAM
                # round-trip latency while the pipeline drains
                for kt in range(nt):
                    r0 = rc + kt * 128
                    pse = psmall.tile([128, 1], F32, tag="sm", name="pse")
                    nc.tensor.matmul(pse[:], e_row[0:1, r0:r0 + 128], id1[:],
                                     start=True, stop=True)
                    nc.vector.tensor_copy(ecol[:, kt:kt + 1], pse[:])
            else:
                nc.sync.dma_start(out=ecol[:, :nt],
                                  in_=escratch[rc:rc + nr]
                                  .rearrange("(t p) -> p t", p=128))
            st["ecol"] = ecol
            # per-batch-item softmax bookkeeping as soon as a batch
            # completes; attn overwrites e_row in place (raw e is already
            # parked in escratch for the context matmul)
            for b in range(rc // N, (rc + nr) // N):
                nc.vector.tensor_reduce(zrow[0:1, b:b + 1],
                                        e_row[0:1, b * N:(b + 1) * N],
                                        axis=mybir.AxisListType.X,
                                        op=mybir.AluOpType.add)
                nc.vector.reciprocal(rzrow[0:1, b:b + 1], zrow[0:1, b:b + 1])
                nc.vector.tensor_scalar_mul(e_row[0:1, b * N:(b + 1) * N],
                                            e_row[0:1, b * N:(b + 1) * N],
                                            rzrow[0:1, b:b + 1])

        def stage_ctx(c):
            rc, nr = CHUNKS[c]
            st = state[c]
            vnat = st["vnat"]
            nt = nr // 128
            ecol = st["ecol"]
            for kt in range(nt):
                kg = rc // 128 + kt
                acol = acol_pool.tile([128, BSH], BF16, name="acol")
                nc.vector.tensor_scalar_mul(acol[:], mask_sb[:, kg, :],
                                            ecol[:, kt:kt + 1])
                for nd in range(NM):
                    nc.tensor.matmul(psum_ctx[nd * 32:(nd + 1) * 32, :],
                                     acol[:],
                                     vnat[:, kt, nd * 512:(nd + 1) * 512],
                                     start=(kg == 0), stop=(kg == NKT - 1),
                                     tile_position=(0, nd * 32))
            state.pop(c)

        NC = len(CHUNKS)
        for c in range(NC + 2):
            if c < NC:
                stage_load(c)
                stage_va(c)
            if 0 <= c - 1 < NC:
                stage_scores(c - 1)
            if 0 <= c - 2 < NC:
                stage_ctx(c - 2)

        # ---- epilogue -------------------------------------------------
        # rz broadcast to all four 32-partition column groups
        prz = psmall.tile([128, 1], F32, tag="sm", name="prz")
        for j in range(NM):
            nc.tensor.matmul(prz[j * 32:(j + 1) * 32, :], rzrow[:], id1[:],
                             start=True, stop=True,
                             tile_position=(0, j * 32))
        nc.vector.tensor_copy(rz_col4[:], prz[:])
        nc.vector.tensor_scalar_mul(ctx_sb[:], psum_ctx[:], rz_col4[:])
        for j in range(NM):
            nc.sync.dma_start(out=out_ctx[:, j * 512:(j + 1) * 512],
                              in_=ctx_sb[j * 32:(j + 1) * 32, :])
        nc.sync.dma_start(out=out_attn, in_=e_row[:])

    nc.compile()
    return nc


def _prepare_in_maps(values, query, W1, b1, W2, b2, v, bv):
    w1b = np.ascontiguousarray(np.asarray(W1) * W1_SCALE).astype(FP8_NP)
    w2b = np.ascontiguousarray(W2).astype(BF16_NP)
    b12 = (np.asarray(b1) + np.asarray(b2)).astype(np.float32)
    b12T = np.ascontiguousarray(b12.reshape(A // 128, 128).T)
    vvb = np.asarray(v).reshape(A, 1).astype(BF16_NP)
    bvv = np.asarray(bv).reshape(1, 1).astype(np.float32)
    # per row-tile masks: column 0 selects rows of batch b0 = r0//N, column 1
    # rows of b1 = (r0+127)//N when the tile straddles a batch boundary
    maskw = np.zeros((128, NKT, BSH), np.float32)
    for kt in range(NKT):
        rows_b = np.arange(kt * 128, kt * 128 + 128) // N
        maskw[np.arange(128), kt, rows_b] = 1.0
    maskw = np.ascontiguousarray(maskw.astype(BF16_NP))

    in_maps = []
    for i in range(NCORES):
        vsh = np.asarray(values[i * BSH:(i + 1) * BSH]).reshape(ROWS, DV)
        qsh = np.asarray(query[i * BSH:(i + 1) * BSH])
        vsh_bf = np.ascontiguousarray(vsh).astype(BF16_NP)
        in_maps.append({
            "vals": vsh_bf,
            "valsT": np.ascontiguousarray(vsh.T).astype(FP8_NP),
            "qT": np.ascontiguousarray(qsh.T).astype(BF16_NP),
            "w1": w1b,
            "w2": w2b,
            "b12T": b12T,
            "vv": vvb,
            "bvv": bvv,
            "maskw": maskw,
        })
    return in_maps


def kernel(values, query, W1, b1, W2, b2, v, bv):
    if "nc" not in _CACHE:
        _CACHE["nc"] = _build_nc()
    nc = _CACHE["nc"]
    in_maps = _prepare_in_maps(values, query, W1, b1, W2, b2, v, bv)
    res = run_bass_kernel_spmd(nc, in_maps, list(range(NCORES)))
    _CACHE["last_result"] = res
    context = np.concatenate(
        [np.asarray(res.results[i]["out_ctx"]) for i in range(NCORES)], axis=0)
    attn = np.concatenate(
        [np.asarray(res.results[i]["out_attn"]) for i in range(NCORES)], axis=0)
    return context.astype(np.float32), attn.astype(np.float32)


# revision 41
# speedup vs baseline: 1.2793x; 1.2793x over previous
"""Additive attention (Bahdanau-style) on 8 TRN2 NeuronCores.

Data-parallel over batch: each core handles 32 of the 256 batch items.
Per core (rows = 32*196 = 6272, Dv = 2048, A = 512):

  vaT   = W1^T @ values^T                                [A, rows]  (PE)
  tanhT = tanh(vaT + qaT[:, b(r)])     (qaT bias folded into ScalarE)
  s     = v^T @ tanhT                                    [1, rows]  (PE)
  e     = exp(s + bv)     (no max subtraction: |s| <= ||v||_1 ~ 11) (ScalarE)
  Z_b   = sum_n e, attn = e/Z        (incremental, per batch item)  (VectorE)
  ctx   = blockdiag(attn)^T @ values  (4x PE column-group packing)

Compute dtype is bf16 (host-cast inputs); accumulation is fp32 in PSUM.
values is supplied in both layouts ([rows, Dv] for the context matmul and
[Dv, rows] for the W1 matmul) as bf16 - same total HBM bytes as one fp32
copy. A 3-stage software pipeline (scores trail the W1 matmul by one chunk,
context by two) keeps the TensorEngine stream dense.
"""

import numpy as np
import ml_dtypes

from contextlib import ExitStack

from concourse import bacc, tile, mybir
from concourse.bass_utils import run_bass_kernel_spmd

F32 = mybir.dt.float32
BF16 = mybir.dt.bfloat16
FP8 = mybir.dt.float8e4
BF16_NP = ml_dtypes.bfloat16
FP8_NP = ml_dtypes.float8_e4m3fn
W1_SCALE = 256.0

NCORES = 8
B = 256
BSH = B // NCORES          # 32 batch items per core
N = 196                    # sequence length
ROWS = BSH * N             # 6272
DV = 2048
A = 512
DQ = 512

CHUNK = 512                # rows per pipeline chunk (4 partition tiles)
NKT = ROWS // 128          # 49
CHUNKS = [(i * CHUNK, min(CHUNK, ROWS - i * CHUNK))
          for i in range((ROWS + CHUNK - 1) // CHUNK)]

_CACHE = {}


def _bsegs(rc, nr):
    """Split chunk rows [rc, rc+nr) at batch-item boundaries.
    Yields (local_start, local_end, b)."""
    out = []
    r = rc
    while r < rc + nr:
        b = r // N
        e = min((b + 1) * N, rc + nr)
        out.append((r - rc, e - rc, b))
        r = e
    return out


def _build_nc():
    nc = bacc.Bacc("TRN2", target_bir_lowering=False, debug=False,
                   num_devices=NCORES)

    vals = nc.dram_tensor("vals", [ROWS, DV], BF16, kind="ExternalInput").ap()
    valsT = nc.dram_tensor("valsT", [DV, ROWS], FP8, kind="ExternalInput").ap()
    qT = nc.dram_tensor("qT", [DQ, BSH], BF16, kind="ExternalInput").ap()
    w1 = nc.dram_tensor("w1", [DV, A], FP8, kind="ExternalInput").ap()
    w2 = nc.dram_tensor("w2", [DQ, A], BF16, kind="ExternalInput").ap()
    b12T = nc.dram_tensor("b12T", [128, A // 128], F32, kind="ExternalInput").ap()
    vv = nc.dram_tensor("vv", [A, 1], BF16, kind="ExternalInput").ap()
    bvv = nc.dram_tensor("bvv", [1, 1], F32, kind="ExternalInput").ap()
    maskw = nc.dram_tensor("maskw", [128, NKT, BSH], BF16,
                           kind="ExternalInput").ap()
    out_ctx = nc.dram_tensor("out_ctx", [BSH, DV], F32, kind="ExternalOutput").ap()
    out_attn = nc.dram_tensor("out_attn", [BSH, N], F32, kind="ExternalOutput").ap()

    TANH = mybir.ActivationFunctionType.Tanh
    EXP = mybir.ActivationFunctionType.Exp
    NM = A // 128   # 4 m-tiles of the attention dim
    NKC = DV // 128  # 16 contraction tiles

    with tile.TileContext(nc) as tc, ExitStack() as ctx:
        consts = ctx.enter_context(tc.tile_pool(name="consts", bufs=1))
        ppersist = ctx.enter_context(tc.tile_pool(name="ppersist", bufs=1, space="PSUM"))
        pva = ctx.enter_context(tc.tile_pool(name="pva", bufs=3, space="PSUM"))
        psmall = ctx.enter_context(tc.tile_pool(name="psmall", bufs=2, space="PSUM"))
        vnat_pool = ctx.enter_context(tc.tile_pool(name="vnat", bufs=3))
        vt_pool = ctx.enter_context(tc.tile_pool(name="vt", bufs=3))
        tanh_pool = ctx.enter_context(tc.tile_pool(name="tanh", bufs=2))
        acol_pool = ctx.enter_context(tc.tile_pool(name="acol", bufs=4))
        ecol_pool = ctx.enter_context(tc.tile_pool(name="ecol", bufs=3))
        pvb_pool = ctx.enter_context(tc.tile_pool(name="pvb", bufs=3))
        dram_pool = ctx.enter_context(tc.tile_pool(name="dram", bufs=1, space="DRAM"))

        # ---- constants / weights (ACT HWDGE ring; small ones first so
        # the query projection and first W1 k-tiles are ready ASAP) ------
        qT_sb = consts.tile([128, DQ // 128, BSH], BF16)
        nc.scalar.dma_start(out=qT_sb[:], in_=qT.rearrange("(k p) b -> p k b", p=128))
        w2_sb = consts.tile([128, DQ // 128, A], BF16)
        nc.scalar.dma_start(out=w2_sb[:], in_=w2.rearrange("(k p) a -> p k a", p=128))
        b12T_sb = consts.tile([128, NM], F32)
        nc.scalar.dma_start(out=b12T_sb[:], in_=b12T)
        w1_sb = consts.tile([128, NKC, A], FP8)
        w1_v = w1.rearrange("(k p) a -> p k a", p=128)
        for q in range(4):
            kq = NKC // 4
            nc.scalar.dma_start(out=w1_sb[:, q * kq:(q + 1) * kq, :],
                                in_=w1_v[:, q * kq:(q + 1) * kq, :])
        v_sb = consts.tile([128, NM, 1], BF16)
        nc.scalar.dma_start(out=v_sb[:], in_=vv.rearrange("(k p) o -> p k o", p=128))
        bv_sb = consts.tile([1, 1], F32)
        nc.scalar.dma_start(out=bv_sb[:], in_=bvv)
        mask_sb = consts.tile([128, NKT, BSH], BF16)
        nc.scalar.dma_start(out=mask_sb[:], in_=maskw)
        id1 = consts.tile([1, 1], F32)
        nc.vector.memset(id1[:], 1.0)
        # warmup fodder: keeps the PE busy (and the HAM un-throttled) while
        # the first weight/value DMAs are still in flight
        wsrc = consts.tile([128, 512], BF16)
        nc.vector.memset(wsrc[:], 0.0)

        e_row = consts.tile([1, ROWS], F32)
        escratch = dram_pool.tile([ROWS], F32)
        ctx_sb = consts.tile([128, DV // NM], F32)
        zrow = consts.tile([1, BSH], F32)
        rzrow = consts.tile([1, BSH], F32)
        rz_col4 = consts.tile([128, 1], F32)
        qaT_sb = consts.tile([128, NM, BSH], F32)

        # context accumulator: column group j holds dv quarter j on
        # partitions [32j, 32j+32)
        psum_ctx = ppersist.tile([128, DV // NM], F32)

        # ---- PE warmup while startup DMAs land ------------------------
        for w in range(10):
            pwarm = psmall.tile([128, 512], F32, tag="sm", name="pwarm")
            nc.tensor.matmul(pwarm[:, 0:384], wsrc[:, 0:128], wsrc[:, 0:384],
                             start=True, stop=True)

        # ---- query projection qaT = W2^T @ q^T + (b1+b2) --------------
        pqa = psmall.tile([128, NM, BSH], F32, tag="sm")
        for m in range(NM):
            for j in range(DQ // 128):
                nc.tensor.matmul(pqa[:, m, :], w2_sb[:, j, m * 128:(m + 1) * 128],
                                 qT_sb[:, j, :], start=(j == 0),
                                 stop=(j == DQ // 128 - 1))
        for m in range(NM):
            nc.vector.tensor_scalar_add(qaT_sb[:, m, :], pqa[:, m, :],
                                        b12T_sb[:, m:m + 1])

        # ---- 3-stage pipelined main loop ------------------------------
        state = {}

        def stage_load(c):
            rc, nr = CHUNKS[c]
            vt = vt_pool.tile([128, NKC, CHUNK], FP8, name="vt")
            vT_v = valsT.rearrange("(k p) r -> p k r", p=128)
            # split so each quarter's matmuls can start as soon as it lands
            nsplit = 4 if c == 0 else 2
            kq = NKC // nsplit
            for q in range(nsplit):
                nc.sync.dma_start(out=vt[:, q * kq:(q + 1) * kq, :nr],
                                  in_=vT_v[:, q * kq:(q + 1) * kq, rc:rc + nr])
            vnat = vnat_pool.tile([128, CHUNK // 128, DV], BF16, name="vnat")
            nc.sync.dma_start(
                out=vnat[:, :nr // 128, :],
                in_=vals[rc:rc + nr, :].rearrange("(t p) d -> p t d", p=128))
            state[c] = {"vt": vt, "vnat": vnat}

        def stage_va(c):
            rc, nr = CHUNKS[c]
            st = state[c]
            vt = st["vt"]
            tanhT = [tanh_pool.tile([128, CHUNK], BF16, tag=f"tanh{m}",
                                    name=f"tanhT{m}") for m in range(NM)]
            for m in range(NM):
                pv = pva.tile([128, CHUNK], F32, name="pv")
                for kp in range(NKC // 2):
                    nc.tensor.matmul(pv[:, :nr],
                                     w1_sb[:, 2 * kp:2 * kp + 2,
                                           m * 128:(m + 1) * 128],
                                     vt[:, 2 * kp:2 * kp + 2, :nr],
                                     start=(kp == 0), stop=(kp == NKC // 2 - 1),
                                     perf_mode=mybir.MatmulPerfMode.DoubleRow)
                for (s0, s1, b) in _bsegs(rc, nr):
                    nc.scalar.activation(tanhT[m][:, s0:s1], pv[:, s0:s1], TANH,
                                         bias=qaT_sb[:, m, b:b + 1],
                                         scale=1.0 / W1_SCALE)
            st["tanhT"] = tanhT

        def stage_scores(c):
            rc, nr = CHUNKS[c]
            st = state[c]
            tanhT = st["tanhT"]
            gs = slice(rc, rc + nr)
            psc = psmall.tile([1, CHUNK], F32, tag="sm", name="psc")
            for j in range(NM):
                nc.tensor.matmul(psc[:, :nr], v_sb[:, j, :], tanhT[j][:, :nr],
                                 start=(j == 0), stop=(j == NM - 1))
            nc.scalar.activation(e_row[0:1, gs], psc[:, :nr], EXP,
                                 bias=bv_sb[0:1, 0:1])
            if c < len(CHUNKS) - 2:
                # park e in DRAM so it can come back partition-major
                nc.sync.dma_start(out=escratch[gs], in_=e_row[0:1, gs])
            nt = nr // 128
            ecol = ecol_pool.tile([128, CHUNK // 128], F32, name="ecol")
            if c >= len(CHUNKS) - 2:
                # tail chunks: PE-transpose e into columns to skip the DRAM
                # round-trip latency while the pipeline drains
                for kt in range(nt):
                    r0 = rc + kt * 128
                    pse = psmall.tile([128, 1], F32, tag="sm", name="pse")
                    nc.tensor.matmul(pse[:], e_row[0:1, r0:r0 + 128], id1[:],
                                     start=True, stop=True)
                    nc.vector.tensor_copy(ecol[:, kt:kt + 1], pse[:])
            else:
                nc.sync.dma_start(out=ecol[:, :nt],
                                  in_=escratch[rc:rc + nr]
                                  .rearrange("(t p) -> p t", p=128))
            st["ecol"] = ecol
            # per-batch-item softmax bookkeeping as soon as a batch
            # completes; attn overwrites e_row in place (raw e is already
            # parked in escratch for the context matmul)
            for b in range(rc // N, (rc + nr) // N):
                nc.vector.tensor_reduce(zrow[0:1, b:b + 1],
                                        e_row[0:1, b * N:(b + 1) * N],
                                        axis=mybir.AxisListType.X,
                                        op=mybir.AluOpType.add)
                nc.vector.reciprocal(rzrow[0:1, b:b + 1], zrow[0:1, b:b + 1])
                nc.vector.tensor_scalar_mul(e_row[0:1, b * N:(b + 1) * N],
                                            e_row[0:1, b * N:(b + 1) * N],
                                            rzrow[0:1, b:b + 1])

        def stage_ctx(c):
            rc, nr = CHUNKS[c]
            st = state[c]
            vnat = st["vnat"]
            nt = nr // 128
            ecol = st["ecol"]
            for kt in range(nt):
                kg = rc // 128 + kt
                acol = acol_pool.tile([128, BSH], BF16, name="acol")
                nc.vector.tensor_scalar_mul(acol[:], mask_sb[:, kg, :],
                                            ecol[:, kt:kt + 1])
                for nd in range(NM):
                    nc.tensor.matmul(psum_ctx[nd * 32:(nd + 1) * 32, :],
                                     acol[:],
                                     vnat[:, kt, nd * 512:(nd + 1) * 512],
                                     start=(kg == 0), stop=(kg == NKT - 1),
                                     tile_position=(0, nd * 32))
            state.pop(c)

        NC = len(CHUNKS)
        for c in range(NC + 2):
            if c < NC:
                stage_load(c)
                stage_va(c)
            if 0 <= c - 1 < NC:
                stage_scores(c - 1)
            if 0 <= c - 2 < NC:
                stage_ctx(c - 2)

        # ---- epilogue -------------------------------------------------
        # rz broadcast to all four 32-partition column groups
        prz = psmall.tile([128, 1], F32, tag="sm", name="prz")
        for j in range(NM):
            nc.tensor.matmul(prz[j * 32:(j + 1) * 32, :], rzrow[:], id1[:],
                             start=True, stop=True,
                             tile_position=(0, j * 32))
        nc.vector.tensor_copy(rz_col4[:], prz[:])
        nc.vector.tensor_scalar_mul(ctx_sb[:], psum_ctx[:], rz_col4[:])
        for j in range(NM):
            nc.sync.dma_start(out=out_ctx[:, j * 512:(j + 1) * 512],
                              in_=ctx_sb[j * 32:(j + 1) * 32, :])
        nc.sync.dma_start(out=out_attn, in_=e_row[:])

    nc.compile()
    return nc


def _prepare_in_maps(values, query, W1, b1, W2, b2, v, bv):
    w1b = np.ascontiguousarray(np.asarray(W1) * W1_SCALE).astype(FP8_NP)
    w2b = np.ascontiguousarray(W2).astype(BF16_NP)
    b12 = (np.asarray(b1) + np.asarray(b2)).astype(np.float32)
    b12T = np.ascontiguousarray(b12.reshape(A // 128, 128).T)
    vvb = np.asarray(v).reshape(A, 1).astype(BF16_NP)
    bvv = np.asarray(bv).reshape(1, 1).astype(np.float32)
    # per row-tile masks: column 0 selects rows of batch b0 = r0//N, column 1
    # rows of b1 = (r0+127)//N when the tile straddles a batch boundary
    maskw = np.zeros((128, NKT, BSH), np.float32)
    for kt in range(NKT):
        rows_b = np.arange(kt * 128, kt * 128 + 128) // N
        maskw[np.arange(128), kt, rows_b] = 1.0
    maskw = np.ascontiguousarray(maskw.astype(BF16_NP))

    in_maps = []
    for i in range(NCORES):
        vsh = np.asarray(values[i * BSH:(i + 1) * BSH]).reshape(ROWS, DV)
        qsh = np.asarray(query[i * BSH:(i + 1) * BSH])
        vsh_bf = np.ascontiguousarray(vsh).astype(BF16_NP)
        in_maps.append({
            "vals": vsh_bf,
            "valsT": np.ascontiguousarray(vsh.T).astype(FP8_NP),
            "qT": np.ascontiguousarray(qsh.T).astype(BF16_NP),
            "w1": w1b,
            "w2": w2b,
            "b12T": b12T,
            "vv": vvb,
            "bvv": bvv,
            "maskw": maskw,
        })
    return in_maps


def kernel(values, query, W1, b1, W2, b2, v, bv):
    values = np.asarray(values, dtype=np.float32)
    query = np.asarray(query, dtype=np.float32)
    W1 = np.asarray(W1, dtype=np.float32)
    b1 = np.asarray(b1, dtype=np.float32)
    W2 = np.asarray(W2, dtype=np.float32)
    b2 = np.asarray(b2, dtype=np.float32)
    v = np.asarray(v, dtype=np.float32)
    bv = np.asarray(bv, dtype=np.float32)
    if "nc" not in _CACHE:
        _CACHE["nc"] = _build_nc()
    nc = _CACHE["nc"]
    in_maps = _prepare_in_maps(values, query, W1, b1, W2, b2, v, bv)
    res = None
    for attempt in range(3):
        try:
            res = run_bass_kernel_spmd(nc, in_maps, list(range(NCORES)))
            break
        except Exception:
            if attempt == 2:
                raise
    _CACHE["last_result"] = res
    context = np.concatenate(
        [np.asarray(res.results[i]["out_ctx"]) for i in range(NCORES)], axis=0)
    attn = np.concatenate(
        [np.asarray(res.results[i]["out_attn"]) for i in range(NCORES)], axis=0)
    return context.astype(np.float32), attn.astype(np.float32)


# revision 42
# speedup vs baseline: 1.4704x; 1.1494x over previous
"""Additive attention (Bahdanau-style) on 8 TRN2 NeuronCores.

Data-parallel over batch: each core handles 32 of the 256 batch items.
Per core (rows = 32*196 = 6272, Dv = 2048, A = 512):

  vaT   = W1^T @ values^T            [A, rows]  (PE, fp8e4m3 DoubleRow)
  tanhT = tanh(vaT/256 + qaT[:, b(r)])   (qaT bias folded into ScalarE)
  s     = v^T @ tanhT                               [1, rows]  (PE, bf16)
  e     = exp(s + bv)    (no max subtraction: |s| <= ||v||_1 ~ 11) (ScalarE)
  Z_b   = sum_n e, attn = e/Z        (incremental, per batch item) (VectorE)
  ctx   = blockdiag(e)^T @ values / Z  (bf16, 4x PE column-group packing)

The dominant matmul runs in fp8e4m3 with DoubleRow (2 contraction tiles
per instruction); W1 is pre-scaled by 256 on the host to clear the e4m3
subnormal floor and the scale is undone inside the tanh activation.
Everything accumulates in fp32 PSUM; measured rel-err ~1e-2 vs the fp32
reference (gate 2e-2). values is supplied in both layouts ([rows, Dv] in
bf16 for the context matmul, [Dv, rows] in fp8 for the W1 matmul). A
3-stage software pipeline (scores trail the W1 matmul by one chunk,
context by two, e transposed to columns via a DRAM round-trip) keeps the
TensorEngine stream dense; the last two chunks transpose e on the PE
instead to shorten the pipeline drain.
"""

import numpy as np
import ml_dtypes

from contextlib import ExitStack

from concourse import bacc, tile, mybir
from concourse.bass_utils import run_bass_kernel_spmd

F32 = mybir.dt.float32
BF16 = mybir.dt.bfloat16
FP8 = mybir.dt.float8e4
BF16_NP = ml_dtypes.bfloat16
FP8_NP = ml_dtypes.float8_e4m3fn
W1_SCALE = 256.0

NCORES = 8
B = 256
BSH = B // NCORES          # 32 batch items per core
N = 196                    # sequence length
ROWS = BSH * N             # 6272
DV = 2048
A = 512
DQ = 512

CHUNK = 512                # rows per pipeline chunk (4 partition tiles)
NKT = ROWS // 128          # 49
CHUNKS = [(i * CHUNK, min(CHUNK, ROWS - i * CHUNK))
          for i in range((ROWS + CHUNK - 1) // CHUNK)]

_CACHE = {}


def _bsegs(rc, nr):
    """Split chunk rows [rc, rc+nr) at batch-item boundaries.
    Yields (local_start, local_end, b)."""
    out = []
    r = rc
    while r < rc + nr:
        b = r // N
        e = min((b + 1) * N, rc + nr)
        out.append((r - rc, e - rc, b))
        r = e
    return out


def _build_nc():
    nc = bacc.Bacc("TRN2", target_bir_lowering=False, debug=False,
                   num_devices=NCORES)

    vals = nc.dram_tensor("vals", [ROWS, DV], BF16, kind="ExternalInput").ap()
    valsT = nc.dram_tensor("valsT", [DV, ROWS], FP8, kind="ExternalInput").ap()
    qT = nc.dram_tensor("qT", [DQ, BSH], BF16, kind="ExternalInput").ap()
    w1 = nc.dram_tensor("w1", [DV, A], FP8, kind="ExternalInput").ap()
    w2 = nc.dram_tensor("w2", [DQ, A], BF16, kind="ExternalInput").ap()
    b12T = nc.dram_tensor("b12T", [128, A // 128], F32, kind="ExternalInput").ap()
    vv = nc.dram_tensor("vv", [A, 1], BF16, kind="ExternalInput").ap()
    bvv = nc.dram_tensor("bvv", [1, 1], F32, kind="ExternalInput").ap()
    maskw = nc.dram_tensor("maskw", [128, NKT, BSH], BF16,
                           kind="ExternalInput").ap()
    out_ctx = nc.dram_tensor("out_ctx", [BSH, DV], F32, kind="ExternalOutput").ap()
    out_attn = nc.dram_tensor("out_attn", [BSH, N], F32, kind="ExternalOutput").ap()

    TANH = mybir.ActivationFunctionType.Tanh
    EXP = mybir.ActivationFunctionType.Exp
    NM = A // 128   # 4 m-tiles of the attention dim
    NKC = DV // 128  # 16 contraction tiles

    with tile.TileContext(nc) as tc, ExitStack() as ctx:
        consts = ctx.enter_context(tc.tile_pool(name="consts", bufs=1))
        ppersist = ctx.enter_context(tc.tile_pool(name="ppersist", bufs=1, space="PSUM"))
        pva = ctx.enter_context(tc.tile_pool(name="pva", bufs=3, space="PSUM"))
        psmall = ctx.enter_context(tc.tile_pool(name="psmall", bufs=2, space="PSUM"))
        vnat_pool = ctx.enter_context(tc.tile_pool(name="vnat", bufs=3))
        vt_pool = ctx.enter_context(tc.tile_pool(name="vt", bufs=3))
        tanh_pool = ctx.enter_context(tc.tile_pool(name="tanh", bufs=2))
        acol_pool = ctx.enter_context(tc.tile_pool(name="acol", bufs=4))
        ecol_pool = ctx.enter_context(tc.tile_pool(name="ecol", bufs=3))
        pvb_pool = ctx.enter_context(tc.tile_pool(name="pvb", bufs=3))
        dram_pool = ctx.enter_context(tc.tile_pool(name="dram", bufs=1, space="DRAM"))

        # ---- constants / weights (ACT HWDGE ring; small ones first so
        # the query projection and first W1 k-tiles are ready ASAP) ------
        qT_sb = consts.tile([128, DQ // 128, BSH], BF16)
        nc.scalar.dma_start(out=qT_sb[:], in_=qT.rearrange("(k p) b -> p k b", p=128))
        w2_sb = consts.tile([128, DQ // 128, A], BF16)
        nc.scalar.dma_start(out=w2_sb[:], in_=w2.rearrange("(k p) a -> p k a", p=128))
        b12T_sb = consts.tile([128, NM], F32)
        nc.scalar.dma_start(out=b12T_sb[:], in_=b12T)
        w1_sb = consts.tile([128, NKC, A], FP8)
        w1_v = w1.rearrange("(k p) a -> p k a", p=128)
        for q in range(4):
            kq = NKC // 4
            nc.scalar.dma_start(out=w1_sb[:, q * kq:(q + 1) * kq, :],
                                in_=w1_v[:, q * kq:(q + 1) * kq, :])
        v_sb = consts.tile([128, NM, 1], BF16)
        nc.scalar.dma_start(out=v_sb[:], in_=vv.rearrange("(k p) o -> p k o", p=128))
        bv_sb = consts.tile([1, 1], F32)
        nc.scalar.dma_start(out=bv_sb[:], in_=bvv)
        mask_sb = consts.tile([128, NKT, BSH], BF16)
        nc.scalar.dma_start(out=mask_sb[:], in_=maskw)
        id1 = consts.tile([1, 1], F32)
        nc.vector.memset(id1[:], 1.0)
        # warmup fodder: keeps the PE busy (and the HAM un-throttled) while
        # the first weight/value DMAs are still in flight
        wsrc = consts.tile([128, 512], BF16)
        nc.vector.memset(wsrc[:], 0.0)

        e_row = consts.tile([1, ROWS], F32)
        escratch = dram_pool.tile([ROWS], F32)
        ctx_sb = consts.tile([128, DV // NM], F32)
        zrow = consts.tile([1, BSH], F32)
        rzrow = consts.tile([1, BSH], F32)
        rz_col4 = consts.tile([128, 1], F32)
        qaT_sb = consts.tile([128, NM, BSH], F32)

        # context accumulator: column group j holds dv quarter j on
        # partitions [32j, 32j+32)
        psum_ctx = ppersist.tile([128, DV // NM], F32)

        # ---- PE warmup while startup DMAs land ------------------------
        for w in range(10):
            pwarm = psmall.tile([128, 512], F32, tag="sm", name="pwarm")
            nc.tensor.matmul(pwarm[:, 0:384], wsrc[:, 0:128], wsrc[:, 0:384],
                             start=True, stop=True)

        # ---- query projection qaT = W2^T @ q^T + (b1+b2) --------------
        pqa = psmall.tile([128, NM, BSH], F32, tag="sm")
        for m in range(NM):
            for j in range(DQ // 128):
                nc.tensor.matmul(pqa[:, m, :], w2_sb[:, j, m * 128:(m + 1) * 128],
                                 qT_sb[:, j, :], start=(j == 0),
                                 stop=(j == DQ // 128 - 1))
        for m in range(NM):
            nc.vector.tensor_scalar_add(qaT_sb[:, m, :], pqa[:, m, :],
                                        b12T_sb[:, m:m + 1])

        # ---- 3-stage pipelined main loop ------------------------------
        state = {}

        def stage_load(c):
            rc, nr = CHUNKS[c]
            vt = vt_pool.tile([128, NKC, CHUNK], FP8, name="vt")
            vT_v = valsT.rearrange("(k p) r -> p k r", p=128)
            # split so each quarter's matmuls can start as soon as it lands
            nsplit = 4 if c == 0 else 2
            kq = NKC // nsplit
            for q in range(nsplit):
                nc.sync.dma_start(out=vt[:, q * kq:(q + 1) * kq, :nr],
                                  in_=vT_v[:, q * kq:(q + 1) * kq, rc:rc + nr])
            vnat = vnat_pool.tile([128, CHUNK // 128, DV], BF16, name="vnat")
            nc.sync.dma_start(
                out=vnat[:, :nr // 128, :],
                in_=vals[rc:rc + nr, :].rearrange("(t p) d -> p t d", p=128))
            state[c] = {"vt": vt, "vnat": vnat}

        def stage_va(c):
            rc, nr = CHUNKS[c]
            st = state[c]
            vt = st["vt"]
            tanhT = [tanh_pool.tile([128, CHUNK], BF16, tag=f"tanh{m}",
                                    name=f"tanhT{m}") for m in range(NM)]
            for m in range(NM):
                pv = pva.tile([128, CHUNK], F32, name="pv")
                for kp in range(NKC // 2):
                    nc.tensor.matmul(pv[:, :nr],
                                     w1_sb[:, 2 * kp:2 * kp + 2,
                                           m * 128:(m + 1) * 128],
                                     vt[:, 2 * kp:2 * kp + 2, :nr],
                                     start=(kp == 0), stop=(kp == NKC // 2 - 1),
                                     perf_mode=mybir.MatmulPerfMode.DoubleRow)
                for (s0, s1, b) in _bsegs(rc, nr):
                    nc.scalar.activation(tanhT[m][:, s0:s1], pv[:, s0:s1], TANH,
                                         bias=qaT_sb[:, m, b:b + 1],
                                         scale=1.0 / W1_SCALE)
            st["tanhT"] = tanhT

        def stage_scores(c):
            rc, nr = CHUNKS[c]
            st = state[c]
            tanhT = st["tanhT"]
            gs = slice(rc, rc + nr)
            psc = psmall.tile([1, CHUNK], F32, tag="sm", name="psc")
            for j in range(NM):
                nc.tensor.matmul(psc[:, :nr], v_sb[:, j, :], tanhT[j][:, :nr],
                                 start=(j == 0), stop=(j == NM - 1))
            nc.scalar.activation(e_row[0:1, gs], psc[:, :nr], EXP,
                                 bias=bv_sb[0:1, 0:1])
            if c < len(CHUNKS) - 2:
                # park e in DRAM so it can come back partition-major
                nc.sync.dma_start(out=escratch[gs], in_=e_row[0:1, gs])
            nt = nr // 128
            ecol = ecol_pool.tile([128, CHUNK // 128], F32, name="ecol")
            if c >= len(CHUNKS) - 2:
                # tail chunks: PE-transpose e into columns to skip the DRAM
                # round-trip latency while the pipeline drains
                for kt in range(nt):
                    r0 = rc + kt * 128
                    pse = psmall.tile([128, 1], F32, tag="sm", name="pse")
                    nc.tensor.matmul(pse[:], e_row[0:1, r0:r0 + 128], id1[:],
                                     start=True, stop=True)
                    nc.vector.tensor_copy(ecol[:, kt:kt + 1], pse[:])
            else:
                nc.sync.dma_start(out=ecol[:, :nt],
                                  in_=escratch[rc:rc + nr]
                                  .rearrange("(t p) -> p t", p=128))
            st["ecol"] = ecol
            # per-batch-item softmax bookkeeping as soon as a batch
            # completes; attn overwrites e_row in place (raw e is already
            # parked in escratch for the context matmul)
            for b in range(rc // N, (rc + nr) // N):
                nc.vector.tensor_reduce(zrow[0:1, b:b + 1],
                                        e_row[0:1, b * N:(b + 1) * N],
                                        axis=mybir.AxisListType.X,
                                        op=mybir.AluOpType.add)
                nc.vector.reciprocal(rzrow[0:1, b:b + 1], zrow[0:1, b:b + 1])
                nc.vector.tensor_scalar_mul(e_row[0:1, b * N:(b + 1) * N],
                                            e_row[0:1, b * N:(b + 1) * N],
                                            rzrow[0:1, b:b + 1])

        def stage_ctx(c):
            rc, nr = CHUNKS[c]
            st = state[c]
            vnat = st["vnat"]
            nt = nr // 128
            ecol = st["ecol"]
            for kt in range(nt):
                kg = rc // 128 + kt
                acol = acol_pool.tile([128, BSH], BF16, name="acol")
                nc.vector.tensor_scalar_mul(acol[:], mask_sb[:, kg, :],
                                            ecol[:, kt:kt + 1])
                for nd in range(NM):
                    nc.tensor.matmul(psum_ctx[nd * 32:(nd + 1) * 32, :],
                                     acol[:],
                                     vnat[:, kt, nd * 512:(nd + 1) * 512],
                                     start=(kg == 0), stop=(kg == NKT - 1),
                                     tile_position=(0, nd * 32))
            state.pop(c)

        NC = len(CHUNKS)
        for c in range(NC + 2):
            if c < NC:
                stage_load(c)
                stage_va(c)
            if 0 <= c - 1 < NC:
                stage_scores(c - 1)
            if 0 <= c - 2 < NC:
                stage_ctx(c - 2)

        # ---- epilogue -------------------------------------------------
        # rz broadcast to all four 32-partition column groups
        prz = psmall.tile([128, 1], F32, tag="sm", name="prz")
        for j in range(NM):
            nc.tensor.matmul(prz[j * 32:(j + 1) * 32, :], rzrow[:], id1[:],
                             start=True, stop=True,
                             tile_position=(0, j * 32))
        nc.vector.tensor_copy(rz_col4[:], prz[:])
        nc.vector.tensor_scalar_mul(ctx_sb[:], psum_ctx[:], rz_col4[:])
        for j in range(NM):
            nc.sync.dma_start(out=out_ctx[:, j * 512:(j + 1) * 512],
                              in_=ctx_sb[j * 32:(j + 1) * 32, :])
        nc.sync.dma_start(out=out_attn, in_=e_row[:])

    nc.compile()
    return nc


def _prepare_in_maps(values, query, W1, b1, W2, b2, v, bv):
    w1b = np.ascontiguousarray(np.asarray(W1) * W1_SCALE).astype(FP8_NP)
    w2b = np.ascontiguousarray(W2).astype(BF16_NP)
    b12 = (np.asarray(b1) + np.asarray(b2)).astype(np.float32)
    b12T = np.ascontiguousarray(b12.reshape(A // 128, 128).T)
    vvb = np.asarray(v).reshape(A, 1).astype(BF16_NP)
    bvv = np.asarray(bv).reshape(1, 1).astype(np.float32)
    # per row-tile masks: column 0 selects rows of batch b0 = r0//N, column 1
    # rows of b1 = (r0+127)//N when the tile straddles a batch boundary
    maskw = np.zeros((128, NKT, BSH), np.float32)
    for kt in range(NKT):
        rows_b = np.arange(kt * 128, kt * 128 + 128) // N
        maskw[np.arange(128), kt, rows_b] = 1.0
    maskw = np.ascontiguousarray(maskw.astype(BF16_NP))

    in_maps = []
    for i in range(NCORES):
        vsh = np.asarray(values[i * BSH:(i + 1) * BSH]).reshape(ROWS, DV)
        qsh = np.asarray(query[i * BSH:(i + 1) * BSH])
        vsh_bf = np.ascontiguousarray(vsh).astype(BF16_NP)
        in_maps.append({
            "vals": vsh_bf,
            "valsT": np.ascontiguousarray(vsh.T).astype(FP8_NP),
            "qT": np.ascontiguousarray(qsh.T).astype(BF16_NP),
            "w1": w1b,
            "w2": w2b,
            "b12T": b12T,
            "vv": vvb,
            "bvv": bvv,
            "maskw": maskw,
        })
    return in_maps


def kernel(values, query, W1, b1, W2, b2, v, bv):
    values = np.asarray(values, dtype=np.float32)
    query = np.asarray(query, dtype=np.float32)
    W1 = np.asarray(W1, dtype=np.float32)
    b1 = np.asarray(b1, dtype=np.float32)
    W2 = np.asarray(W2, dtype=np.float32)
    b2 = np.asarray(b2, dtype=np.float32)
    v = np.asarray(v, dtype=np.float32)
    bv = np.asarray(bv, dtype=np.float32)
    if "nc" not in _CACHE:
        _CACHE["nc"] = _build_nc()
    nc = _CACHE["nc"]
    in_maps = _prepare_in_maps(values, query, W1, b1, W2, b2, v, bv)
    res = None
    for attempt in range(3):
        try:
            res = run_bass_kernel_spmd(nc, in_maps, list(range(NCORES)))
            break
        except Exception:
            if attempt == 2:
                raise
    _CACHE["last_result"] = res
    context = np.concatenate(
        [np.asarray(res.results[i]["out_ctx"]) for i in range(NCORES)], axis=0)
    attn = np.concatenate(
        [np.asarray(res.results[i]["out_attn"]) for i in range(NCORES)], axis=0)
    return context.astype(np.float32), attn.astype(np.float32)


# revision 44
# speedup vs baseline: 1.4980x; 1.0188x over previous
"""Additive attention (Bahdanau-style) on 8 TRN2 NeuronCores.

Data-parallel over batch: each core handles 32 of the 256 batch items.
Per core (rows = 32*196 = 6272, Dv = 2048, A = 512):

  vaT   = W1^T @ values^T            [A, rows]  (PE, fp8e4m3 DoubleRow)
  tanhT = tanh(vaT/256 + qaT[:, b(r)])   (qaT bias folded into ScalarE)
  s     = v^T @ tanhT                               [1, rows]  (PE, bf16)
  e     = exp(s + bv)    (no max subtraction: |s| <= ||v||_1 ~ 11) (ScalarE)
  Z_b   = sum_n e, attn = e/Z        (incremental, per batch item) (VectorE)
  ctx   = blockdiag(e)^T @ values / Z  (bf16, 4x PE column-group packing)

The dominant matmul runs in fp8e4m3 with DoubleRow (2 contraction tiles
per instruction); W1 is pre-scaled by 256 on the host to clear the e4m3
subnormal floor and the scale is undone inside the tanh activation.
Everything accumulates in fp32 PSUM; measured rel-err ~1e-2 vs the fp32
reference (gate 2e-2). values is supplied in both layouts ([rows, Dv] in
bf16 for the context matmul, [Dv, rows] in fp8 for the W1 matmul). A
3-stage software pipeline (scores trail the W1 matmul by one chunk,
context by two, e transposed to columns via a DRAM round-trip) keeps the
TensorEngine stream dense; the last two chunks transpose e on the PE
instead to shorten the pipeline drain.
"""

import numpy as np
import ml_dtypes

from contextlib import ExitStack

from concourse import bacc, tile, mybir
from concourse.bass_utils import run_bass_kernel_spmd

F32 = mybir.dt.float32
BF16 = mybir.dt.bfloat16
FP8 = mybir.dt.float8e4
BF16_NP = ml_dtypes.bfloat16
FP8_NP = ml_dtypes.float8_e4m3fn
W1_SCALE = 256.0

NCORES = 8
B = 256
BSH = B // NCORES          # 32 batch items per core
N = 196                    # sequence length
ROWS = BSH * N             # 6272
DV = 2048
A = 512
DQ = 512

CHUNK = 512                # rows per pipeline chunk (4 partition tiles)
NKT = ROWS // 128          # 49
CHUNKS = [(i * CHUNK, min(CHUNK, ROWS - i * CHUNK))
          for i in range((ROWS + CHUNK - 1) // CHUNK)]

_CACHE = {}


def _bsegs(rc, nr):
    """Split chunk rows [rc, rc+nr) at batch-item boundaries.
    Yields (local_start, local_end, b)."""
    out = []
    r = rc
    while r < rc + nr:
        b = r // N
        e = min((b + 1) * N, rc + nr)
        out.append((r - rc, e - rc, b))
        r = e
    return out


def _build_nc():
    nc = bacc.Bacc("TRN2", target_bir_lowering=False, debug=False,
                   num_devices=NCORES)

    vals = nc.dram_tensor("vals", [ROWS, DV], BF16, kind="ExternalInput").ap()
    valsT = nc.dram_tensor("valsT", [DV, ROWS], FP8, kind="ExternalInput").ap()
    qT = nc.dram_tensor("qT", [DQ, BSH], BF16, kind="ExternalInput").ap()
    w1 = nc.dram_tensor("w1", [DV, A], FP8, kind="ExternalInput").ap()
    w2 = nc.dram_tensor("w2", [DQ, A], BF16, kind="ExternalInput").ap()
    b12T = nc.dram_tensor("b12T", [128, A // 128], F32, kind="ExternalInput").ap()
    vv = nc.dram_tensor("vv", [A, 1], BF16, kind="ExternalInput").ap()
    bvv = nc.dram_tensor("bvv", [1, 1], F32, kind="ExternalInput").ap()
    maskw = nc.dram_tensor("maskw", [128, NKT, BSH], BF16,
                           kind="ExternalInput").ap()
    out_ctx = nc.dram_tensor("out_ctx", [BSH, DV], F32, kind="ExternalOutput").ap()
    out_attn = nc.dram_tensor("out_attn", [BSH, N], F32, kind="ExternalOutput").ap()

    TANH = mybir.ActivationFunctionType.Tanh
    EXP = mybir.ActivationFunctionType.Exp
    NM = A // 128   # 4 m-tiles of the attention dim
    NKC = DV // 128  # 16 contraction tiles

    with tile.TileContext(nc) as tc, ExitStack() as ctx:
        consts = ctx.enter_context(tc.tile_pool(name="consts", bufs=1))
        ppersist = ctx.enter_context(tc.tile_pool(name="ppersist", bufs=1, space="PSUM"))
        pva = ctx.enter_context(tc.tile_pool(name="pva", bufs=3, space="PSUM"))
        psmall = ctx.enter_context(tc.tile_pool(name="psmall", bufs=2, space="PSUM"))
        vnat_pool = ctx.enter_context(tc.tile_pool(name="vnat", bufs=3))
        vt_pool = ctx.enter_context(tc.tile_pool(name="vt", bufs=3))
        tanh_pool = ctx.enter_context(tc.tile_pool(name="tanh", bufs=2))
        acol_pool = ctx.enter_context(tc.tile_pool(name="acol", bufs=4))
        ecol_pool = ctx.enter_context(tc.tile_pool(name="ecol", bufs=3))
        pvb_pool = ctx.enter_context(tc.tile_pool(name="pvb", bufs=3))
        dram_pool = ctx.enter_context(tc.tile_pool(name="dram", bufs=1, space="DRAM"))

        # ---- constants / weights (ACT HWDGE ring; small ones first so
        # the query projection and first W1 k-tiles are ready ASAP) ------
        qT_sb = consts.tile([128, DQ // 128, BSH], BF16)
        nc.scalar.dma_start(out=qT_sb[:], in_=qT.rearrange("(k p) b -> p k b", p=128))
        w2_sb = consts.tile([128, DQ // 128, A], BF16)
        nc.scalar.dma_start(out=w2_sb[:], in_=w2.rearrange("(k p) a -> p k a", p=128))
        b12T_sb = consts.tile([128, NM], F32)
        nc.scalar.dma_start(out=b12T_sb[:], in_=b12T)
        w1_sb = consts.tile([128, NKC, A], FP8)
        w1_v = w1.rearrange("(k p) a -> p k a", p=128)
        for q in range(4):
            kq = NKC // 4
            nc.scalar.dma_start(out=w1_sb[:, q * kq:(q + 1) * kq, :],
                                in_=w1_v[:, q * kq:(q + 1) * kq, :])
        v_sb = consts.tile([128, NM, 1], BF16)
        nc.scalar.dma_start(out=v_sb[:], in_=vv.rearrange("(k p) o -> p k o", p=128))
        bv_sb = consts.tile([1, 1], F32)
        nc.scalar.dma_start(out=bv_sb[:], in_=bvv)
        mask_sb = consts.tile([128, NKT, BSH], BF16)
        nc.scalar.dma_start(out=mask_sb[:], in_=maskw)
        id1 = consts.tile([1, 1], F32)
        nc.vector.memset(id1[:], 1.0)
        # warmup fodder: keeps the PE busy (and the HAM un-throttled) while
        # the first weight/value DMAs are still in flight
        wsrc = consts.tile([128, 512], BF16)
        nc.vector.memset(wsrc[:], 0.0)

        e_row = consts.tile([1, ROWS], F32)
        escratch = dram_pool.tile([ROWS], F32)
        ctx_sb = consts.tile([128, DV // NM], F32)
        zrow = consts.tile([1, BSH], F32)
        rzrow = consts.tile([1, BSH], F32)
        rz_col4 = consts.tile([128, 1], F32)
        qaT_sb = consts.tile([128, NM, BSH], F32)

        # context accumulator: column group j holds dv quarter j on
        # partitions [32j, 32j+32)
        psum_ctx = ppersist.tile([128, DV // NM], F32)

        # ---- PE warmup while startup DMAs land ------------------------
        for w in range(10):
            pwarm = psmall.tile([128, 512], F32, tag="sm", name="pwarm")
            nc.tensor.matmul(pwarm[:, 0:384], wsrc[:, 0:128], wsrc[:, 0:384],
                             start=True, stop=True)

        # ---- query projection qaT = W2^T @ q^T + (b1+b2) --------------
        pqa = psmall.tile([128, NM, BSH], F32, tag="sm")
        for m in range(NM):
            for j in range(DQ // 128):
                nc.tensor.matmul(pqa[:, m, :], w2_sb[:, j, m * 128:(m + 1) * 128],
                                 qT_sb[:, j, :], start=(j == 0),
                                 stop=(j == DQ // 128 - 1))
        for m in range(NM):
            nc.vector.tensor_scalar_add(qaT_sb[:, m, :], pqa[:, m, :],
                                        b12T_sb[:, m:m + 1])

        # ---- 3-stage pipelined main loop ------------------------------
        state = {}

        def stage_load(c):
            rc, nr = CHUNKS[c]
            vt = vt_pool.tile([128, NKC, CHUNK], FP8, name="vt")
            vT_v = valsT.rearrange("(k p) r -> p k r", p=128)
            # chunk 0 finely split so the first matmuls start early; later
            # chunks in halves (fewer issues on the sync ring)
            nsplit = 4 if c == 0 else 2
            kq = NKC // nsplit
            for q in range(nsplit):
                nc.sync.dma_start(out=vt[:, q * kq:(q + 1) * kq, :nr],
                                  in_=vT_v[:, q * kq:(q + 1) * kq, rc:rc + nr])
            vnat = vnat_pool.tile([128, CHUNK // 128, DV], BF16, name="vnat")
            nc.sync.dma_start(
                out=vnat[:, :nr // 128, :],
                in_=vals[rc:rc + nr, :].rearrange("(t p) d -> p t d", p=128))
            state[c] = {"vt": vt, "vnat": vnat}

        def stage_va(c):
            rc, nr = CHUNKS[c]
            st = state[c]
            vt = st["vt"]
            tanhT = [tanh_pool.tile([128, CHUNK], BF16, tag=f"tanh{m}",
                                    name=f"tanhT{m}") for m in range(NM)]
            for m in range(NM):
                pv = pva.tile([128, CHUNK], F32, name="pv")
                for kp in range(NKC // 2):
                    nc.tensor.matmul(pv[:, :nr],
                                     w1_sb[:, 2 * kp:2 * kp + 2,
                                           m * 128:(m + 1) * 128],
                                     vt[:, 2 * kp:2 * kp + 2, :nr],
                                     start=(kp == 0), stop=(kp == NKC // 2 - 1),
                                     perf_mode=mybir.MatmulPerfMode.DoubleRow)
                for (s0, s1, b) in _bsegs(rc, nr):
                    nc.scalar.activation(tanhT[m][:, s0:s1], pv[:, s0:s1], TANH,
                                         bias=qaT_sb[:, m, b:b + 1],
                                         scale=1.0 / W1_SCALE)
            st["tanhT"] = tanhT

        def stage_scores(c):
            rc, nr = CHUNKS[c]
            st = state[c]
            tanhT = st["tanhT"]
            gs = slice(rc, rc + nr)
            psc = psmall.tile([1, CHUNK], F32, tag="sm", name="psc")
            for j in range(NM):
                nc.tensor.matmul(psc[:, :nr], v_sb[:, j, :], tanhT[j][:, :nr],
                                 start=(j == 0), stop=(j == NM - 1))
            nc.scalar.activation(e_row[0:1, gs], psc[:, :nr], EXP,
                                 bias=bv_sb[0:1, 0:1])
            if c < len(CHUNKS) - 2:
                # park e in DRAM so it can come back partition-major
                nc.sync.dma_start(out=escratch[gs], in_=e_row[0:1, gs])
            nt = nr // 128
            ecol = ecol_pool.tile([128, CHUNK // 128], F32, name="ecol")
            if c >= len(CHUNKS) - 2:
                # tail chunks: PE-transpose e into columns to skip the DRAM
                # round-trip latency while the pipeline drains
                for kt in range(nt):
                    r0 = rc + kt * 128
                    pse = psmall.tile([128, 1], F32, tag="sm", name="pse")
                    nc.tensor.matmul(pse[:], e_row[0:1, r0:r0 + 128], id1[:],
                                     start=True, stop=True)
                    nc.vector.tensor_copy(ecol[:, kt:kt + 1], pse[:])
            else:
                nc.sync.dma_start(out=ecol[:, :nt],
                                  in_=escratch[rc:rc + nr]
                                  .rearrange("(t p) -> p t", p=128))
            st["ecol"] = ecol
            # per-batch-item softmax bookkeeping as soon as a batch
            # completes; attn overwrites e_row in place (raw e is already
            # parked in escratch for the context matmul)
            for b in range(rc // N, (rc + nr) // N):
                nc.vector.tensor_reduce(zrow[0:1, b:b + 1],
                                        e_row[0:1, b * N:(b + 1) * N],
                                        axis=mybir.AxisListType.X,
                                        op=mybir.AluOpType.add)
                nc.vector.reciprocal(rzrow[0:1, b:b + 1], zrow[0:1, b:b + 1])
                nc.vector.tensor_scalar_mul(e_row[0:1, b * N:(b + 1) * N],
                                            e_row[0:1, b * N:(b + 1) * N],
                                            rzrow[0:1, b:b + 1])

        def stage_ctx(c):
            rc, nr = CHUNKS[c]
            st = state[c]
            vnat = st["vnat"]
            nt = nr // 128
            ecol = st["ecol"]
            for kt in range(nt):
                kg = rc // 128 + kt
                acol = acol_pool.tile([128, BSH], BF16, name="acol")
                nc.vector.tensor_scalar_mul(acol[:], mask_sb[:, kg, :],
                                            ecol[:, kt:kt + 1])
                for nd in range(NM):
                    nc.tensor.matmul(psum_ctx[nd * 32:(nd + 1) * 32, :],
                                     acol[:],
                                     vnat[:, kt, nd * 512:(nd + 1) * 512],
                                     start=(kg == 0), stop=(kg == NKT - 1),
                                     tile_position=(0, nd * 32))
            state.pop(c)

        NC = len(CHUNKS)
        for c in range(NC + 2):
            if c < NC:
                stage_load(c)
                stage_va(c)
            if 0 <= c - 1 < NC:
                stage_scores(c - 1)
            if 0 <= c - 2 < NC:
                stage_ctx(c - 2)

        # ---- epilogue -------------------------------------------------
        # rz broadcast to all four 32-partition column groups
        prz = psmall.tile([128, 1], F32, tag="sm", name="prz")
        for j in range(NM):
            nc.tensor.matmul(prz[j * 32:(j + 1) * 32, :], rzrow[:], id1[:],
                             start=True, stop=True,
                             tile_position=(0, j * 32))
        nc.vector.tensor_copy(rz_col4[:], prz[:])
        nc.vector.tensor_scalar_mul(ctx_sb[:], psum_ctx[:], rz_col4[:])
        for j in range(NM):
            nc.sync.dma_start(out=out_ctx[:, j * 512:(j + 1) * 512],
                              in_=ctx_sb[j * 32:(j + 1) * 32, :])
        nc.sync.dma_start(out=out_attn, in_=e_row[:])

    nc.compile()
    return nc


def _prepare_in_maps(values, query, W1, b1, W2, b2, v, bv):
    w1b = np.ascontiguousarray(np.asarray(W1) * W1_SCALE).astype(FP8_NP)
    w2b = np.ascontiguousarray(W2).astype(BF16_NP)
    b12 = (np.asarray(b1) + np.asarray(b2)).astype(np.float32)
    b12T = np.ascontiguousarray(b12.reshape(A // 128, 128).T)
    vvb = np.asarray(v).reshape(A, 1).astype(BF16_NP)
    bvv = np.asarray(bv).reshape(1, 1).astype(np.float32)
    # per row-tile masks: column 0 selects rows of batch b0 = r0//N, column 1
    # rows of b1 = (r0+127)//N when the tile straddles a batch boundary
    maskw = np.zeros((128, NKT, BSH), np.float32)
    for kt in range(NKT):
        rows_b = np.arange(kt * 128, kt * 128 + 128) // N
        maskw[np.arange(128), kt, rows_b] = 1.0
    maskw = np.ascontiguousarray(maskw.astype(BF16_NP))

    in_maps = []
    for i in range(NCORES):
        vsh = np.asarray(values[i * BSH:(i + 1) * BSH]).reshape(ROWS, DV)
        qsh = np.asarray(query[i * BSH:(i + 1) * BSH])
        vsh_bf = np.ascontiguousarray(vsh).astype(BF16_NP)
        in_maps.append({
            "vals": vsh_bf,
            "valsT": np.ascontiguousarray(vsh.T).astype(FP8_NP),
            "qT": np.ascontiguousarray(qsh.T).astype(BF16_NP),
            "w1": w1b,
            "w2": w2b,
            "b12T": b12T,
            "vv": vvb,
            "bvv": bvv,
            "maskw": maskw,
        })
    return in_maps


def kernel(values, query, W1, b1, W2, b2, v, bv):
    values = np.asarray(values, dtype=np.float32)
    query = np.asarray(query, dtype=np.float32)
    W1 = np.asarray(W1, dtype=np.float32)
    b1 = np.asarray(b1, dtype=np.float32)
    W2 = np.asarray(W2, dtype=np.float32)
    b2 = np.asarray(b2, dtype=np.float32)
    v = np.asarray(v, dtype=np.float32)
    bv = np.asarray(bv, dtype=np.float32)
    if "nc" not in _CACHE:
        _CACHE["nc"] = _build_nc()
    nc = _CACHE["nc"]
    in_maps = _prepare_in_maps(values, query, W1, b1, W2, b2, v, bv)
    res = None
    for attempt in range(3):
        try:
            res = run_bass_kernel_spmd(nc, in_maps, list(range(NCORES)))
            break
        except Exception:
            if attempt == 2:
                raise
    _CACHE["last_result"] = res
    context = np.concatenate(
        [np.asarray(res.results[i]["out_ctx"]) for i in range(NCORES)], axis=0)
    attn = np.concatenate(
        [np.asarray(res.results[i]["out_attn"]) for i in range(NCORES)], axis=0)
    return context.astype(np.float32), attn.astype(np.float32)
